# revision 8
# baseline (speedup 1.0000x reference)
"""Trainium2 Bass kernel: CorDBN (ZCA channel whitening) over X[128, 64, 56, 56].

Math: with x = X viewed as [C=64, m=B*H*W], the op is
    out = wm @ ((x - mean) / std)
where std is the per-channel (ddof=1) std + 1e-5, sigma = eps*I + corr/m and
wm = sigma^{-1/2}.  This is a per-column affine map out = A @ x + b with
    A = wm @ diag(1/std),    b = -wm @ (mean/std).

Plan (8 cores, data-parallel over batch, 16 batches per core):
  phase 1: DMA two-batch tiles [128, 3136] into SBUF (kept resident),
           PE-transpose 128-column slices, accumulate the augmented Gram
           matrix [S | row-sums] (bf16 operands, fp32 PSUM accumulate).
  stats:   AllReduce the [65, 64] Gram across cores; compute sigma and run
           a Newton-Schulz iteration on-device for wm = sigma^{-1/2}
           (sigma ~ I + small for this regime, so NS converges fast);
           build block-diag lhsT [A^T | A^T] and the bias vector.
  phase 2: one matmul per [128, 512] chunk against the resident tiles,
           bias added during the PSUM->SBUF copy (split ACT/DVE), DMA out.
"""
import numpy as np

import concourse.bass as bass
import concourse.tile as tile
from concourse import mybir
from concourse.bass_utils import run_bass_kernel_spmd
from concourse.vector_clock import ScopedClock

# ---------------- problem constants (hardcoded: must be self-contained) ----
B, C, H, W = 128, 64, 56, 56
HW = H * W                      # 3136
N_CORES = 8
B_LOC = B // N_CORES            # 16 batches per core
PAIRS = B_LOC // 2              # 8 two-batch tiles per core
M_TOT = B * HW                  # 401408
EPS = 1e-3
EPS_BN = 1e-5
NS_ITERS = 5
F32 = mybir.dt.float32
BF16 = mybir.dt.bfloat16

TCH = 128                       # transpose chunk width (phase 1)
N_FULL = HW // TCH              # 24
REM = HW - N_FULL * TCH         # 64
OCH = 512                       # phase-2 output chunk width
P2_CHUNKS = [(i * OCH, OCH) for i in range(HW // OCH)] + [
    (HW - HW % OCH, HW % OCH)
]  # 6 x 512 + 1 x 64


# ---------------- old-walrus workaround: 1 sync wait per instruction -------
# This walrus build rejects instructions carrying more than one sem wait
# ("Too many sync wait commands").  Split: excess waits move onto fresh
# same-engine nops placed immediately before the instruction.
_MAXW = 1

_orig_commit_and_lower = tile.TileContext._commit_and_lower


def _commit_and_lower_split(self, inst, bb, old_bb_map, bb_to_exit_bb):
    si = inst.sync_info
    if si is not None and len(si.on_wait) > _MAXW:
        waits = list(si.on_wait)
        excess = waits[:-_MAXW]
        del si.on_wait[:len(waits) - _MAXW]
        eng = self.nc.engines[inst.engine]
        for i in range(0, len(excess), _MAXW):
            nop = eng.nop(nofuse=True, hint="split_wait")
            nop.ins.sync_info = mybir.SyncInfo(
                on_wait=list(excess[i:i + _MAXW]), on_update=[]
            )
    return _orig_commit_and_lower(self, inst, bb, old_bb_map, bb_to_exit_bb)


tile.TileContext._commit_and_lower = _commit_and_lower_split


def _drain_and_barrier_split(self, tick_clock, wait_clock):
    MAXW = _MAXW
    probe = self.nc.sync.drain()
    wait_clock.add_sem_waits(probe.ins, ScopedClock({None: tick_clock.global_clock}))
    if probe.ins.sync_info is None:
        probe.ins.sync_info = mybir.SyncInfo(on_wait=[], on_update=[])
    n = len(probe.ins.sync_info.on_wait)
    del probe.ins.sync_info.on_wait[MAXW:]
    for start in range(MAXW, n, MAXW):
        extra = self.nc.sync.drain()
        wait_clock.add_sem_waits(
            extra.ins, ScopedClock({None: tick_clock.global_clock})
        )
        si = extra.ins.sync_info
        del si.on_wait[start + MAXW:]
        del si.on_wait[:start]
    self.nc.all_engine_barrier()
    popped = self.nc._tile_sem_poison_stack.pop()
    assert popped is self._sem_poison
    self.nc.clear_and_free_semaphores(list(self.sems.allocated().values()))
    self.nc.all_engine_barrier()


tile.TileContext._drain_and_barrier = _drain_and_barrier_split


def build_bass(repeat: int = 1):
    nc = bass.Bass("TRN2", target_bir_lowering=False, debug=False,
                   num_devices=N_CORES)
    X = nc.dram_tensor("X", [B_LOC, C, HW], F32, kind="ExternalInput").ap()
    OUT = nc.dram_tensor("OUT", [B_LOC, C, HW], F32, kind="ExternalOutput").ap()
    IDENT = nc.dram_tensor("IDENT", [128, 128], F32, kind="ExternalInput").ap()
    EYE3 = nc.dram_tensor("EYE3", [C, C], F32, kind="ExternalInput").ap()
    EPSEYE = nc.dram_tensor("EPSEYE", [C, C], F32, kind="ExternalInput").ap()

    cc_in = nc.dram_tensor("cc_in", [C + 1, C], F32)
    cc_out = nc.dram_tensor("cc_out", [C + 1, C], F32, addr_space="Shared")

    with tile.TileContext(nc) as tc:
        with (
            tc.tile_pool(name="const", bufs=1) as cpool,
            tc.tile_pool(name="xres", bufs=1) as xpool,
            tc.tile_pool(name="tsb", bufs=1) as tsbpool,
            tc.tile_pool(name="small", bufs=1) as spool,
            tc.tile_pool(name="outs", bufs=2) as opool,
            tc.tile_pool(name="tp_ps", bufs=1, space="PSUM") as tppool,
            tc.tile_pool(name="acc_ps", bufs=1, space="PSUM") as accpool,
            tc.tile_pool(name="stat_ps", bufs=2, space="PSUM") as stpool,
            tc.tile_pool(name="p2_ps", bufs=3, space="PSUM") as p2pool,
        ):
            ident_sb = cpool.tile([128, 128], F32, tag="ident")
            nc.sync.dma_start(out=ident_sb[:], in_=IDENT)
            eye3_sb = cpool.tile([C, C], F32, tag="eye3")
            nc.sync.dma_start(out=eye3_sb[:], in_=EYE3)
            epseye_sb = cpool.tile([C, C], F32, tag="epseye")
            nc.sync.dma_start(out=epseye_sb[:], in_=EPSEYE)
            eyec = ident_sb[0:C, 0:C]

            # transposed-chunk staging tiles (bf16), manual ring of 3.
            # layout per tile: cols 0-63 half-A data, col 64 ones,
            # cols 65-128 half-B data, col 129 ones.
            tsb_tiles = []
            for i in range(3):
                t = tsbpool.tile([128, 130], BF16, tag=f"tsb{i}")
                nc.vector.memset(t[:, 64:65], 1.0)
                nc.vector.memset(t[:, 129:130], 1.0)
                tsb_tiles.append(t)

            for _rep in range(repeat):
                run_one_pass(nc, tc, cpool, xpool, tsbpool, spool, opool, tppool,
                             accpool, stpool, p2pool,
                             X, OUT, cc_in, cc_out,
                             ident_sb, eye3_sb, epseye_sb, eyec, tsb_tiles)
    return nc


def run_one_pass(nc, tc, cpool, xpool, tsbpool, spool, opool, tppool, accpool,
                 stpool, p2pool, X, OUT, cc_in, cc_out,
                 ident_sb, eye3_sb, epseye_sb, eyec, tsb_tiles):
    if True:
        if True:
            # augmented Gram accumulator: rows 0-63 = S, row 64 = column sums
            s_psum = accpool.tile([C + 1, C], F32, tag="sacc")

            xt = [
                xpool.tile([128, HW], F32, tag=f"xt{p}", name=f"xt{p}")
                for p in range(PAIRS)
            ]

            # ---------------- phase 1 ----------------
            chunks = [(k * TCH, TCH) for k in range(N_FULL)] + [(N_FULL * TCH, REM)]
            n_mm = 0
            total_mm = PAIRS * len(chunks) * 2
            for p in range(PAIRS):
                nc.sync.dma_start(
                    out=xt[p][:],
                    in_=X[2 * p:2 * p + 2].rearrange("b c s -> (b c) s"),
                )
                for ki, (o, w) in enumerate(chunks):
                    tp = tppool.tile([128, 128], F32, tag=f"tp{ki % 2}")
                    nc.tensor.transpose(tp[0:w, :], xt[p][:, o:o + w], ident_sb[:])
                    tsb = tsb_tiles[ki % 3]
                    src = tp[0:w, :].rearrange("p (g c) -> p g c", c=64)
                    dst = tsb[0:w, 0:130].rearrange("p (g c) -> p g c", c=65)[:, :, 0:64]
                    if ki % 2 == 0:
                        nc.vector.tensor_copy(dst, src)
                    else:
                        nc.scalar.copy(dst, src)
                    for h in range(2):
                        nc.tensor.matmul(
                            s_psum[:],
                            lhsT=tsb[0:w, 65 * h:65 * h + 65],
                            rhs=tsb[0:w, 65 * h:65 * h + 64],
                            start=(n_mm == 0),
                            stop=(n_mm == total_mm - 1),
                        )
                        n_mm += 1

            # ---------------- stats + AllReduce ----------------
            g_loc = spool.tile([C + 1, C], F32, tag="gloc")
            nc.scalar.copy(g_loc[:], s_psum[:])
            d_in = nc.sync.dma_start(out=cc_in.ap(), in_=g_loc[:])
            coll = nc.gpsimd.collective_compute(
                "AllReduce",
                mybir.AluOpType.add,
                replica_groups=[list(range(N_CORES))],
                ins=[cc_in.ap()],
                outs=[cc_out.ap()],
            )
            g = spool.tile([C + 1, C], F32, tag="g")
            d_out = nc.sync.dma_start(out=g[:], in_=cc_out.ap())
            from concourse.tile_rust import add_dep_helper
            add_dep_helper(coll.ins, d_in.ins, reason="collective after input dma")
            add_dep_helper(d_out.ins, coll.ins, reason="output dma after collective")

            # mean (in place on row 64 of g)
            nc.vector.tensor_scalar_mul(g[C:C + 1, :], g[C:C + 1, :], 1.0 / M_TOT)
            # cov = S - m * outer(mean, mean)
            outer_ps = stpool.tile([C, C], F32, tag="stat")
            nc.tensor.matmul(outer_ps[:], lhsT=g[C:C + 1, :], rhs=g[C:C + 1, :],
                             start=True, stop=True)
            mouter = spool.tile([C, C], F32, tag="mouter")
            nc.scalar.activation(mouter[:], outer_ps[:],
                                 mybir.ActivationFunctionType.Copy,
                                 scale=float(M_TOT))
            cov = spool.tile([C, C], F32, tag="cov")
            nc.vector.tensor_sub(cov[:], g[0:C, :], mouter[:])
            # per-channel std / rstd
            masked = spool.tile([C, C], F32, tag="masked")
            nc.vector.tensor_tensor(masked[:], cov[:], eyec, mybir.AluOpType.mult)
            var = spool.tile([C, 1], F32, tag="var")
            nc.vector.tensor_reduce(var[:], masked[:], mybir.AxisListType.X,
                                    mybir.AluOpType.add)
            stdv = spool.tile([C, 1], F32, tag="stdv")
            nc.scalar.activation(stdv[:], var[:], mybir.ActivationFunctionType.Sqrt,
                                 scale=1.0 / (M_TOT - 1))
            nc.vector.tensor_scalar_add(stdv[:], stdv[:], EPS_BN)
            rstd = spool.tile([C, 1], F32, tag="rstd")
            nc.vector.reciprocal(rstd[:], stdv[:])
            # sigma = eps*I + diag(rstd) cov diag(rstd) / m
            b1 = spool.tile([C, C], F32, tag="b1")
            nc.vector.tensor_scalar_mul(b1[:], cov[:], rstd[:, 0:1])
            b1t_ps = stpool.tile([C, C], F32, tag="stat")
            nc.tensor.transpose(b1t_ps[:], b1[:], eyec)
            b2 = spool.tile([C, C], F32, tag="b2")
            nc.scalar.activation(b2[:], b1t_ps[:],
                                 mybir.ActivationFunctionType.Copy,
                                 scale=1.0 / M_TOT)
            nc.vector.tensor_scalar_mul(b2[:], b2[:], rstd[:, 0:1])
            sigma = spool.tile([C, C], F32, tag="sigma")
            nc.vector.tensor_add(sigma[:], b2[:], epseye_sb[:])

            # Newton-Schulz: Y0=sigma, Z0=I;  T=3I-ZY; Y<-0.5*Y@T; Z<-0.5*T@Z
            t1 = spool.tile([C, C], F32, tag="ns_t0")
            nc.vector.tensor_sub(t1[:], eye3_sb[:], sigma[:])
            y = spool.tile([C, C], F32, tag="ns_y0")
            y_ps = stpool.tile([C, C], F32, tag="stat")
            nc.tensor.matmul(y_ps[:], lhsT=sigma[:], rhs=t1[:], start=True, stop=True)
            nc.scalar.activation(y[:], y_ps[:], mybir.ActivationFunctionType.Copy,
                                 scale=0.5)
            z = spool.tile([C, C], F32, tag="ns_z0")
            nc.scalar.mul(z[:], t1[:], 0.5)
            for k in range(1, NS_ITERS):
                p_ps = stpool.tile([C, C], F32, tag="stat")
                nc.tensor.matmul(p_ps[:], lhsT=z[:], rhs=y[:], start=True, stop=True)
                tk = spool.tile([C, C], F32, tag=f"ns_t{k}")
                nc.vector.tensor_sub(tk[:], eye3_sb[:], p_ps[:])
                zn = spool.tile([C, C], F32, tag=f"ns_z{k}")
                z_ps = stpool.tile([C, C], F32, tag="stat")
                nc.tensor.matmul(z_ps[:], lhsT=tk[:], rhs=z[:], start=True, stop=True)
                nc.scalar.activation(zn[:], z_ps[:],
                                     mybir.ActivationFunctionType.Copy, scale=0.5)
                if k < NS_ITERS - 1:
                    yn = spool.tile([C, C], F32, tag=f"ns_y{k}")
                    yn_ps = stpool.tile([C, C], F32, tag="stat")
                    nc.tensor.matmul(yn_ps[:], lhsT=y[:], rhs=tk[:],
                                     start=True, stop=True)
                    nc.scalar.activation(yn[:], yn_ps[:],
                                         mybir.ActivationFunctionType.Copy,
                                         scale=0.5)
                    y = yn
                z = zn
            wm = z

            # A^T = diag(rstd) @ wm ; block-diag lhsT; bias
            at = spool.tile([C, C], F32, tag="at")
            nc.vector.tensor_scalar_mul(at[:], wm[:], rstd[:, 0:1])
            bd = cpool.tile([128, 128], F32, tag="bd")
            nc.vector.memset(bd[:], 0.0)
            nc.sync.dma_start(out=bd[0:C, 0:C], in_=at[:])
            nc.sync.dma_start(out=bd[C:2 * C, C:2 * C], in_=at[:])

            mcol = spool.tile([C, 1], F32, tag="mcol")
            nc.sync.dma_start(out=mcol[:], in_=g[C:C + 1, :])
            v = spool.tile([C, 1], F32, tag="v")
            nc.vector.tensor_tensor(v[:], mcol[:], rstd[:], mybir.AluOpType.mult)
            bias_ps = stpool.tile([C, 1], F32, tag="stat")
            nc.tensor.matmul(bias_ps[:], lhsT=wm[:], rhs=v[:], start=True, stop=True)
            bias_sb = spool.tile([C, 1], F32, tag="bias")
            nc.scalar.activation(bias_sb[:], bias_ps[:],
                                 mybir.ActivationFunctionType.Copy, scale=-1.0)
            bias2 = spool.tile([128, 1], F32, tag="bias2")
            nc.sync.dma_start(out=bias2[0:C, :], in_=bias_sb[:])
            nc.sync.dma_start(out=bias2[C:2 * C, :], in_=bias_sb[:])

            # ---------------- phase 2 ----------------
            for p in range(PAIRS):
                osb = opool.tile([128, HW], F32, tag="osb")
                for ci, (o, w) in enumerate(P2_CHUNKS):
                    po = p2pool.tile([128, OCH], F32, tag="p2")
                    nc.tensor.matmul(po[:, 0:w], lhsT=bd[:], rhs=xt[p][:, o:o + w],
                                     start=True, stop=True)
                    if ci % 2 == 0:
                        nc.scalar.activation(osb[:, o:o + w], po[:, 0:w],
                                             mybir.ActivationFunctionType.Identity,
                                             bias=bias2[:, 0:1], scale=1.0)
                    else:
                        nc.vector.tensor_scalar_add(osb[:, o:o + w], po[:, 0:w],
                                                    bias2[:, 0:1])
                nc.sync.dma_start(
                    out=OUT[2 * p:2 * p + 2].rearrange("b c s -> (b c) s"),
                    in_=osb[:],
                )


_NC_CACHE = None


def _get_nc():
    global _NC_CACHE
    if _NC_CACHE is None:
        _NC_CACHE = build_bass()
    return _NC_CACHE


def kernel(X: np.ndarray) -> np.ndarray:
    assert X.shape == (B, C, H, W) and X.dtype == np.float32
    nc = _get_nc()
    ident = np.eye(128, dtype=np.float32)
    eye3 = 3.0 * np.eye(C, dtype=np.float32)
    epseye = EPS * np.eye(C, dtype=np.float32)
    xr = np.ascontiguousarray(X.reshape(B, C, HW))
    in_maps = [
        {
            "X": xr[i * B_LOC:(i + 1) * B_LOC],
            "IDENT": ident,
            "EYE3": eye3,
            "EPSEYE": epseye,
        }
        for i in range(N_CORES)
    ]
    res = run_bass_kernel_spmd(nc, in_maps, core_ids=list(range(N_CORES)))
    out = np.concatenate([res.results[i]["OUT"] for i in range(N_CORES)], axis=0)
    return out.reshape(B, C, H, W)
